# revision 4
# baseline (speedup 1.0000x reference)
"""Bidirectional minGRU (nn_MinGRU2) Trainium2 Bass kernel.

Full input x: [16, 512, 4096] f32. Channel layout per batch:
    0:128    forward h        128:256  forward g
    256:384  backward h       384:512  backward g
Output [16, 256, 4096]: out[:, 0:128] = forward minGRU, out[:, 128:256] =
backward minGRU (scanned right-to-left over L).

The log-space reference reduces to the direct linear recurrence per
(b, channel) lane:
    sig  = sigmoid(g);  coef = sigmoid(-g);  v = h * sig
    y[t] = coef[t] * y[t-1] + v[t]
which maps to one DVE tensor_tensor_scan per [128-lane, L-chunk] tile, with
ACT computing both sigmoids and DVE the multiply. The backward direction
runs the same scan through reversed (negative-stride) access patterns, so
no explicit flip pass is needed.

Sharding: fully data-parallel over batch — 16 batches / 8 cores = 2 per
core; every (b, lane) recurrence is independent and L stays contiguous.
"""
import numpy as np

import concourse.bacc as bacc
import concourse.mybir as mybir
import concourse.tile as tile
from concourse.bass_utils import run_bass_kernel_spmd

B, H, L = 16, 512, 4096
N_CORES = 8
B_PC = B // N_CORES  # batches per core

P = 128
F32 = mybir.dt.float32
MULT = mybir.AluOpType.mult
ADD = mybir.AluOpType.add
SIGMOID = mybir.ActivationFunctionType.Sigmoid

CHUNK = 2048
BUFS = 3
OUT_BUFS = 5


def _emit(tc: tile.TileContext, x, out, chunk=CHUNK, bufs=BUFS, out_bufs=OUT_BUFS):
    nc = tc.nc
    n_chunks = L // chunk
    # streams: (batch, direction); direction 0 = forward, 1 = backward
    streams = [(b, d) for b in range(B_PC) for d in (0, 1)]
    carries = {s: None for s in streams}

    # out tiles live across a chunk boundary (the next chunk's scan reads the
    # carry column), so with S streams in flight up to S+1 must coexist —
    # fewer slots can cycle with engine program order and deadlock.
    with tc.tile_pool(name="io", bufs=bufs) as io, \
         tc.tile_pool(name="mid", bufs=bufs) as mid, \
         tc.tile_pool(name="op", bufs=out_bufs) as op:
        for k in range(n_chunks):
            for (b, d) in streams:
                # forward walks L ascending, backward descending
                l0 = k * chunk if d == 0 else L - (k + 1) * chunk
                sl = slice(l0, l0 + chunk)

                # one DMA: the stream's h- and g-quarters (256 adjacent
                # channels) -> [128 part, 2, chunk]
                in_t = io.tile([P, 2, chunk], F32, tag="in")
                src = x[b, d * 256:(d + 1) * 256, sl]
                nc.sync.dma_start(
                    out=in_t, in_=src.rearrange("(q p) l -> p q l", p=P))

                h_ap = in_t[:, 0, :]
                g_ap = in_t[:, 1, :]

                sig = mid.tile([P, chunk], F32, tag="sig")
                nc.scalar.activation(sig, g_ap, SIGMOID)
                coef = mid.tile([P, chunk], F32, tag="coef")
                nc.scalar.activation(coef, g_ap, SIGMOID, scale=-1.0)
                v = mid.tile([P, chunk], F32, tag="v")
                nc.vector.tensor_tensor(out=v, in0=h_ap, in1=sig, op=MULT)

                out_t = op.tile([P, chunk], F32, tag="out")
                init = carries[(b, d)]
                if init is None:
                    init = 0.0
                if d == 0:
                    nc.vector.tensor_tensor_scan(
                        out=out_t, data0=coef, data1=v, initial=init,
                        op0=MULT, op1=ADD)
                    carries[(b, d)] = out_t[:, chunk - 1:chunk]
                else:
                    nc.vector.tensor_tensor_scan(
                        out=out_t[:, ::-1], data0=coef[:, ::-1],
                        data1=v[:, ::-1], initial=init,
                        op0=MULT, op1=ADD)
                    carries[(b, d)] = out_t[:, 0:1]

                # store on SWDGE (gpsimd) so store triggers (which wait on the
                # scan) don't block later load issue on the SP HWDGE ring
                nc.gpsimd.dma_start(out=out[b, d * P:(d + 1) * P, sl], in_=out_t)


_NC_CACHE = {}


def build(n_repeat=1, **emit_kwargs):
    key = (n_repeat, tuple(sorted(emit_kwargs.items())))
    if key not in _NC_CACHE:
        nc = bacc.Bacc("TRN2", target_bir_lowering=False, debug=False)
        x = nc.dram_tensor("x", [B_PC, H, L], F32, kind="ExternalInput")
        out = nc.dram_tensor("out", [B_PC, H // 2, L], F32, kind="ExternalOutput")
        with tile.TileContext(nc) as tc:
            for _ in range(n_repeat):
                _emit(tc, x.ap(), out.ap(), **emit_kwargs)
        nc.compile()
        _NC_CACHE[key] = nc
    return _NC_CACHE[key]


def kernel(x: np.ndarray):
    assert x.shape == (B, H, L) and x.dtype == np.float32
    nc = build()
    in_maps = [
        {"x": np.ascontiguousarray(x[i * B_PC:(i + 1) * B_PC])}
        for i in range(N_CORES)
    ]
    res = run_bass_kernel_spmd(nc, in_maps, core_ids=list(range(N_CORES)))
    return np.concatenate([r["out"] for r in res.results], axis=0)


# revision 11
# speedup vs baseline: 2947.8661x; 2947.8661x over previous
"""Bidirectional minGRU (nn_MinGRU2) Trainium2 Bass kernel.

Full input x: [16, 512, 4096] f32. Channel layout per batch:
    0:128    forward h        128:256  forward g
    256:384  backward h       384:512  backward g
Output [16, 256, 4096]: out[:, 0:128] = forward minGRU, out[:, 128:256] =
backward minGRU (scanned right-to-left over L).

The log-space reference reduces to the direct linear recurrence per
(b, channel) lane:
    sig  = sigmoid(g);  coef = sigmoid(-g);  v = h * sig
    y[t] = coef[t] * y[t-1] + v[t]
which maps to one DVE tensor_tensor_scan per [128-lane, L-chunk] tile, with
ACT computing both sigmoids and DVE the multiply. The backward direction
runs the same scan through reversed (negative-stride) access patterns, so
no explicit flip pass is needed.

Sharding: fully data-parallel over batch — 16 batches / 8 cores = 2 per
core; every (b, lane) recurrence is independent and L stays contiguous.
"""
import numpy as np

import concourse.bacc as bacc
import concourse.mybir as mybir
import concourse.tile as tile
from concourse.bass_utils import run_bass_kernel_spmd

B, H, L = 16, 512, 4096
N_CORES = 8
B_PC = B // N_CORES  # batches per core

P = 128
F32 = mybir.dt.float32
MULT = mybir.AluOpType.mult
ADD = mybir.AluOpType.add
SIGMOID = mybir.ActivationFunctionType.Sigmoid

CHUNK = 2048
BUFS = 3
IN_BUFS = 4
OUT_BUFS = 6


def _emit(tc: tile.TileContext, x, out, chunk=CHUNK, bufs=BUFS, out_bufs=OUT_BUFS,
          store_eng=0, in_bufs=IN_BUFS):
    nc = tc.nc
    n_chunks = L // chunk
    # streams: (batch, direction); direction 0 = forward, 1 = backward
    streams = [(b, d) for b in range(B_PC) for d in (0, 1)]
    carries = {s: None for s in streams}

    # out tiles live across a chunk boundary (the next chunk's scan reads the
    # carry column), so with S streams in flight up to S+1 must coexist —
    # fewer slots can cycle with engine program order and deadlock.
    with tc.tile_pool(name="io", bufs=in_bufs) as io, \
         tc.tile_pool(name="mid", bufs=bufs) as mid, \
         tc.tile_pool(name="op", bufs=out_bufs) as op:
        for k in range(n_chunks):
            for (b, d) in streams:
                # forward walks L ascending, backward descending
                l0 = k * chunk if d == 0 else L - (k + 1) * chunk
                sl = slice(l0, l0 + chunk)

                # one DMA: the stream's h- and g-quarters (256 adjacent
                # channels) -> [128 part, 2, chunk]
                in_t = io.tile([P, 2, chunk], F32, tag="in")
                src = x[b, d * 256:(d + 1) * 256, sl]
                nc.sync.dma_start(
                    out=in_t, in_=src.rearrange("(q p) l -> p q l", p=P))

                h_ap = in_t[:, 0, :]
                g_ap = in_t[:, 1, :]

                sig = mid.tile([P, chunk], F32, tag="sig")
                nc.scalar.activation(sig, g_ap, SIGMOID)
                coef = mid.tile([P, chunk], F32, tag="coef")
                nc.scalar.activation(coef, g_ap, SIGMOID, scale=-1.0)
                v = mid.tile([P, chunk], F32, tag="v")
                nc.vector.tensor_tensor(out=v, in0=h_ap, in1=sig, op=MULT)

                out_t = op.tile([P, chunk], F32, tag="out")
                init = carries[(b, d)]
                if init is None:
                    init = 0.0
                if d == 0:
                    nc.vector.tensor_tensor_scan(
                        out=out_t, data0=coef, data1=v, initial=init,
                        op0=MULT, op1=ADD)
                    carries[(b, d)] = out_t[:, chunk - 1:chunk]
                else:
                    nc.vector.tensor_tensor_scan(
                        out=out_t[:, ::-1], data0=coef[:, ::-1],
                        data1=v[:, ::-1], initial=init,
                        op0=MULT, op1=ADD)
                    carries[(b, d)] = out_t[:, 0:1]

                # store on SWDGE (gpsimd) so store triggers (which wait on the
                # scan) don't block later load issue on the SP HWDGE ring
                store = (nc.gpsimd, nc.scalar, nc.sync)[store_eng]
                store.dma_start(out=out[b, d * P:(d + 1) * P, sl], in_=out_t)


_NC_CACHE = {}


def build(n_repeat=1, **emit_kwargs):
    key = (n_repeat, tuple(sorted(emit_kwargs.items())))
    if key not in _NC_CACHE:
        nc = bacc.Bacc("TRN2", target_bir_lowering=False, debug=False)
        x = nc.dram_tensor("x", [B_PC, H, L], F32, kind="ExternalInput")
        out = nc.dram_tensor("out", [B_PC, H // 2, L], F32, kind="ExternalOutput")
        with tile.TileContext(nc) as tc:
            for _ in range(n_repeat):
                _emit(tc, x.ap(), out.ap(), **emit_kwargs)
        nc.compile()
        _NC_CACHE[key] = nc
    return _NC_CACHE[key]


def kernel(x: np.ndarray):
    assert x.shape == (B, H, L) and x.dtype == np.float32
    nc = build()
    in_maps = [
        {"x": np.ascontiguousarray(x[i * B_PC:(i + 1) * B_PC])}
        for i in range(N_CORES)
    ]
    res = run_bass_kernel_spmd(nc, in_maps, core_ids=list(range(N_CORES)))
    return np.concatenate([r["out"] for r in res.results], axis=0)
